# revision 1
# baseline (speedup 1.0000x reference)
"""Trainium2 Bass kernel for nn_Attention_13864154431876.

Dense transformer attention block: QKV projection + RoPE + causal GQA
attention (32 q heads, 8 kv heads, head_dim 128) + output projection.
B=2, S=2048, D=4096, start_pos=0 (cache fully overwritten).

Sharding (8 NeuronCores, tensor parallel by attention heads):
  - each core owns 4 q-heads and 1 kv-head (wq/wk/wv output-dim shards)
  - x is replicated (shipped pre-transposed as x^T so the contraction dim
    lands on partitions)
  - after attention, an on-chip AllToAll redistributes attn^T from
    head-sharded to token-sharded; each core then multiplies its 512-token
    slab against the full wo and the host concatenates the 8 slabs.

All matmuls run in float32r (hardware fast-fp32: operands rounded to
11-bit mantissa, exact fp32 accumulate) at 1 cycle/row.
"""
import sys

sys.path.insert(0, "/root/.axon_site/_ro/trn_rl_repo")

import numpy as np

import concourse.bass as bass
import concourse.mybir as mybir
import concourse.tile as tile
from concourse import bacc
from concourse.bass_utils import run_bass_kernel_spmd

F32 = mybir.dt.float32
F32R = mybir.dt.float32r
AF = mybir.ActivationFunctionType
ALU = mybir.AluOpType

N_CORES = 8
B, S, D = 2, 2048, 4096
H, KH, HD = 32, 8, 128
MS = 2048                     # max_seq_len (cache length)
BS = B * S                    # flattened tokens, b-major
HPC = H // N_CORES            # q-heads per core = 4
QF = HPC * HD                 # per-core q-feature width = 512
TB = 512                      # token block
NTB = BS // TB                # 8 token blocks
QBPB = S // TB                # 4 q-blocks per batch element
KC = D // 128                 # 32 contraction chunks
JCB = S // 128                # 16 j-chunks per batch element
SCALE = 1.0 / np.sqrt(HD)
TOKS_PER_CORE = BS // N_CORES  # 512


def round_fp32r(x: np.ndarray) -> np.ndarray:
    """Round fp32 -> fp32r bits (11-bit mantissa, round-to-nearest-even)."""
    u = np.ascontiguousarray(x, dtype=np.float32).view(np.uint32)
    lsb = (u >> 12) & 1
    return ((u + np.uint32(0x7FF) + lsb) & np.uint32(0xFFFFF000)).view(np.float32)


def build_attn_nc(mock_collectives=False):
    nc = bacc.Bacc("TRN2", target_bir_lowering=False, debug=False,
                   num_devices=N_CORES)

    # ---- DRAM I/O ----------------------------------------------------
    xt_d = nc.dram_tensor("xt", [D, BS], F32R, kind="ExternalInput").ap()
    wq_d = nc.dram_tensor("wq", [D, QF], F32R, kind="ExternalInput").ap()
    wk_d = nc.dram_tensor("wk", [D, HD], F32R, kind="ExternalInput").ap()
    wv_d = nc.dram_tensor("wv", [D, HD], F32R, kind="ExternalInput").ap()
    wo_d = nc.dram_tensor("wo", [D, D], F32R, kind="ExternalInput").ap()
    cos_d = nc.dram_tensor("cosT", [HD, S], F32, kind="ExternalInput").ap()
    sin_d = nc.dram_tensor("sinT", [HD, S], F32, kind="ExternalInput").ap()
    mask_d = nc.dram_tensor("maskd", [128, 4, TB], F32, kind="ExternalInput").ap()
    rot_d = nc.dram_tensor("rotm", [HD, HD], F32R, kind="ExternalInput").ap()
    ident_d = nc.dram_tensor("ident", [128, 128], F32, kind="ExternalInput").ap()
    ones_d = nc.dram_tensor("ones128", [128, 128], F32R, kind="ExternalInput").ap()
    y_d = nc.dram_tensor("y", [TOKS_PER_CORE, D], F32, kind="ExternalOutput").ap()

    # internal DRAM for the two per-batch-element AllToAlls.
    # attn_locX rows are chunk-major: chunk j (512 rows) = my 512 head-feats
    # for 256-token group j of batch element X. After A2A, attn_gX rows are
    # global head-feats for MY 256-token slab of batch element X.
    HTB = TB // 2  # 256
    QTB = TB // 4  # 128
    attn_loc = [nc.dram_tensor("attn_loc0", [BS, HTB], F32R),
                nc.dram_tensor("attn_loc1a", [BS, QTB], F32R),
                nc.dram_tensor("attn_loc1b", [BS, QTB], F32R)]
    attn_g = [nc.dram_tensor("attn_g0", [D, HTB], F32R),
              nc.dram_tensor("attn_g1a", [D, QTB], F32R),
              nc.dram_tensor("attn_g1b", [D, QTB], F32R)]

    with tile.TileContext(nc) as tc:
        from concourse.tile_rust import add_dep_helper

        # ---- pool stack (bottom-up; closed LIFO) --------------------
        # persistB + tier2 live until the very end; persistA + transients
        # are closed right after the last projection so phase-3 prefetch
        # DMAs can reuse their SBUF while the attention tail computes.
        persistB_cm = tc.tile_pool(name="persistB", bufs=1)
        persistB = persistB_cm.__enter__()
        mask_sb = persistB.tile([128, 4, TB], F32, name="mask_sb")
        rot_sb = persistB.tile([HD, HD], F32R, name="rot_sb")
        ident_sb = persistB.tile([128, 128], F32, name="ident_sb")
        ones_sb = persistB.tile([128, 128], F32R, name="ones_sb")
        kt_sb = [persistB.tile([HD, S], F32R, name=f"kt{b}_sb") for b in range(B)]
        v_sb = [persistB.tile([128, JCB, HD], F32R, name=f"v{b}_sb") for b in range(B)]
        nc.sync.dma_start(mask_sb[:], mask_d[:])
        nc.sync.dma_start(rot_sb[:], rot_d[:])
        nc.sync.dma_start(ident_sb[:], ident_d[:])
        nc.sync.dma_start(ones_sb[:], ones_d[:])

        tier2 = []

        def pool_t2(*a, **kw):
            cm = tc.tile_pool(*a, **kw)
            p = cm.__enter__()
            tier2.append(cm)
            return p

        qtp = pool_t2(name="qtp", bufs=5)
        tmpp = pool_t2(name="tmpp", bufs=2)
        ptp = pool_t2(name="ptp", bufs=3)
        denp = pool_t2(name="denp", bufs=2)
        recbp = pool_t2(name="recbp", bufs=1)
        atp = pool_t2(name="atp", bufs=2)
        asbp = pool_t2(name="asbp", bufs=2)

        persistA_cm = tc.tile_pool(name="persistA", bufs=1)
        persistA = persistA_cm.__enter__()
        wq_sb = persistA.tile([128, KC, QF], F32R, name="wq_sb")
        wk_sb = persistA.tile([128, KC, HD], F32R, name="wk_sb")
        wv_sb = persistA.tile([128, KC, HD], F32R, name="wv_sb")
        cos_sb = persistA.tile([HD, S], F32, name="cos_sb")
        sin_sb = persistA.tile([HD, S], F32, name="sin_sb")
        nc.sync.dma_start(wq_sb[:], wq_d.rearrange("(kc p) n -> p kc n", p=128))
        nc.sync.dma_start(wk_sb[:], wk_d.rearrange("(kc p) n -> p kc n", p=128))
        nc.sync.dma_start(wv_sb[:], wv_d.rearrange("(kc p) n -> p kc n", p=128))
        nc.sync.dma_start(cos_sb[:], cos_d[:])
        nc.sync.dma_start(sin_sb[:], sin_d[:])

        trans = []

        def pool_tr(*a, **kw):
            cm = tc.tile_pool(*a, **kw)
            p = cm.__enter__()
            trans.append(cm)
            return p

        xtp = pool_tr(name="xtp", bufs=3)
        qrawp = pool_tr(name="qrawp", bufs=2)
        vtrawp = pool_tr(name="vtrawp", bufs=2)
        pp_cm = tc.tile_pool(name="pp", bufs=6, space="PSUM")
        pp = pp_cm.__enter__()
        ps_cm = tc.tile_pool(name="ps", bufs=1, space="PSUM")
        ps = ps_cm.__enter__()
        pa_cm = tc.tile_pool(name="pa", bufs=1, space="PSUM")
        pa = pa_cm.__enter__()

        cc_last_write = {}

        def emit_attention(tb, ps_pool, pa_pool, qt_tiles, delay_on=None):
            """Generator: attention for token block tb. Yields between steps.

            delay_on: instruction the first score matmul must wait for
            (used to start the eager blocks no earlier than the concurrent
            collective's trigger, so they cover its DMA burst)."""
            b, qb = tb // QBPB, tb % QBPB
            njc = (qb + 1) * 4
            grp = 0 if tb < QBPB else (1 if tb < 6 else 2)
            for h in range(HPC):
                denacc = denp.tile([128, TB], F32, name="denacc", tag="den")
                denf = denp.tile([128, TB], F32R, name="denf", tag="den")
                aps = pa_pool.tile([128, TB], F32, name="aps", tag="aps")
                for jc in range(njc):
                    sps = ps_pool.tile([128, TB], F32, name="sps", tag="sps")
                    mm = nc.tensor.matmul(
                        sps[:], kt_sb[b][:, jc * 128:(jc + 1) * 128],
                        qt_tiles[h][:], start=True, stop=True,
                        skip_group_check=True)
                    if delay_on is not None:
                        add_dep_helper(mm.ins, delay_on,
                                       reason="cover collective dma burst")
                        delay_on = None
                    r = jc - qb * 4
                    pt = ptp.tile([128, TB], F32R, name="pt", tag="pt")
                    if r >= 0:
                        praw = tmpp.tile([128, TB], F32, name="praw", tag="tmp")
                        nc.scalar.activation(praw[:], sps[:], AF.Exp)
                        nc.vector.tensor_tensor(pt[:], praw[:],
                                                mask_sb[:, r, :], ALU.mult)
                    else:
                        nc.scalar.activation(pt[:], sps[:], AF.Exp)
                    # denominator accumulation (final add rounds to f32r)
                    if jc == 0:
                        nc.vector.tensor_copy(denacc[:], pt[:].bitcast(F32))
                    elif jc == njc - 1:
                        nc.vector.tensor_tensor(denf[:], denacc[:],
                                                pt[:].bitcast(F32), ALU.add)
                    else:
                        nc.vector.tensor_tensor(denacc[:], denacc[:],
                                                pt[:].bitcast(F32), ALU.add)
                    nc.tensor.matmul(
                        aps[:], v_sb[b][:, jc, :], pt[:],
                        start=(jc == 0), stop=(jc == njc - 1),
                        skip_group_check=True)
                    yield
                # free the attn accumulator bank early; colsum+broadcast in
                # one ones-matmul; fast reciprocal
                asb = asbp.tile([128, TB], F32, name="asb", tag="asb")
                nc.vector.tensor_copy(asb[:], aps[:])
                denb = ps_pool.tile([128, TB], F32, name="denb", tag="sps")
                nc.tensor.matmul(denb[:], ones_sb[:], denf[:],
                                 start=True, stop=True, skip_group_check=True)
                recipb = recbp.tile([128, TB], F32, name="recipb", tag="recb")
                nc.vector.reciprocal_approx_fast(recipb[:], denb[:])
                attn_t = atp.tile([128, TB], F32R, name="attn_t", tag="attn_t")
                nc.vector.tensor_tensor(attn_t[:], asb[:], recipb[:], ALU.mult)
                if grp == 0:
                    for half in range(2):
                        d = nc.sync.dma_start(
                            attn_loc[0].ap()[
                                (2 * tb + half) * 512 + h * 128:
                                (2 * tb + half) * 512 + (h + 1) * 128, :],
                            attn_t[:, half * HTB:(half + 1) * HTB])
                else:
                    lb = (tb - 4) % 2
                    for qt4 in range(4):
                        d = nc.sync.dma_start(
                            attn_loc[grp].ap()[
                                (4 * lb + qt4) * 512 + h * 128:
                                (4 * lb + qt4) * 512 + (h + 1) * 128, :],
                            attn_t[:, qt4 * QTB:(qt4 + 1) * QTB])
                cc_last_write[grp] = d.ins
                yield

        def drive(gen, n):
            if gen is None:
                return None
            for _ in range(n):
                try:
                    next(gen)
                except StopIteration:
                    return None
            return gen

        def emit_a2a(g):
            if mock_collectives:
                nc.sync.dma_start(attn_g[g].ap()[:], attn_loc[g].ap()[:])
            else:
                nc.gpsimd.collective_compute(
                    "AllToAll", ALU.bypass,
                    replica_groups=[list(range(N_CORES))],
                    ins=[attn_loc[g].ap().opt()],
                    outs=[attn_g[g].ap().opt()],
                )

        prev_gen = None
        prev_steps = 0
        for tb in range(NTB):
            b, qb = tb // QBPB, tb % QBPB
            s0 = qb * TB
            per_kc = max(1, -(-prev_steps // KC))  # ceil
            # ---- projections for tb, interleaved with attention(tb-1)
            qps = [pp.tile([128, TB], F32, name=f"qps{h}", tag="proj")
                   for h in range(HPC)]
            kps = pp.tile([128, TB], F32, name="kps", tag="proj")
            vtps = pp.tile([128, TB], F32, name="vtps", tag="proj")
            for kc in range(KC):
                xt_t = xtp.tile([128, TB], F32R, name="xt_t", tag="xt")
                nc.sync.dma_start(
                    xt_t[:], xt_d[kc * 128:(kc + 1) * 128,
                                  tb * TB:(tb + 1) * TB])
                for h in range(HPC):
                    nc.tensor.matmul(
                        qps[h][:], wq_sb[:, kc, h * 128:(h + 1) * 128],
                        xt_t[:], start=(kc == 0), stop=(kc == KC - 1),
                        skip_group_check=True)
                nc.tensor.matmul(kps[:], wk_sb[:, kc, :], xt_t[:],
                                 start=(kc == 0), stop=(kc == KC - 1),
                                 skip_group_check=True)
                nc.tensor.matmul(vtps[:], wv_sb[:, kc, :], xt_t[:],
                                 start=(kc == 0), stop=(kc == KC - 1),
                                 skip_group_check=True)
                prev_gen = drive(prev_gen, per_kc)

            # ---- drains + RoPE + V transpose ------------------------
            qt_tiles = []
            for h in range(HPC):
                qraw = qrawp.tile([128, TB], F32R, name="qraw", tag="qraw")
                nc.vector.tensor_copy(qraw[:], qps[h][:])
                rotps = ps.tile([128, TB], F32, name="rotps", tag="sps")
                nc.tensor.matmul(rotps[:], rot_sb[:], qraw[:],
                                 start=True, stop=True, skip_group_check=True)
                tcos = tmpp.tile([128, TB], F32, name="tcos", tag="tmp")
                nc.vector.tensor_tensor(tcos[:], qraw[:].bitcast(F32),
                                        cos_sb[:, s0:s0 + TB], ALU.mult)
                tsin = tmpp.tile([128, TB], F32, name="tsin", tag="tmp")
                nc.vector.tensor_tensor(tsin[:], rotps[:],
                                        sin_sb[:, s0:s0 + TB], ALU.mult)
                qt = qtp.tile([128, TB], F32R, name="qt", tag="qt")
                nc.vector.tensor_tensor(qt[:], tcos[:], tsin[:], ALU.add)
                qt_tiles.append(qt)
                prev_gen = drive(prev_gen, 1)
            # K
            kraw = qrawp.tile([128, TB], F32R, name="kraw", tag="qraw")
            nc.vector.tensor_copy(kraw[:], kps[:])
            rotps = ps.tile([128, TB], F32, name="rotpsk", tag="sps")
            nc.tensor.matmul(rotps[:], rot_sb[:], kraw[:],
                             start=True, stop=True, skip_group_check=True)
            tcos = tmpp.tile([128, TB], F32, name="tcosk", tag="tmp")
            nc.vector.tensor_tensor(tcos[:], kraw[:].bitcast(F32),
                                    cos_sb[:, s0:s0 + TB], ALU.mult)
            tsin = tmpp.tile([128, TB], F32, name="tsink", tag="tmp")
            nc.vector.tensor_tensor(tsin[:], rotps[:],
                                    sin_sb[:, s0:s0 + TB], ALU.mult)
            nc.vector.tensor_tensor(kt_sb[b][:, s0:s0 + TB], tcos[:],
                                    tsin[:], ALU.add)
            # V: drain V^T then transpose 4x [128,128]
            vtraw = vtrawp.tile([128, TB], F32, name="vtraw", tag="vtraw")
            nc.vector.tensor_copy(vtraw[:], vtps[:])
            vtr = pp.tile([128, TB], F32, name="vtr", tag="proj")
            for t4 in range(4):
                nc.tensor.transpose(vtr[:, t4 * 128:(t4 + 1) * 128],
                                    vtraw[:, t4 * 128:(t4 + 1) * 128],
                                    ident_sb[:])
            nc.vector.tensor_copy(
                v_sb[b].rearrange("p jc d -> p (jc d)")[:, s0:s0 + TB],
                vtr[:])
            prev_gen = drive(prev_gen, 10 ** 9)  # flush any leftovers
            if tb in (4, 6):
                # previous collective group fully written: fire its A2A and
                # run this block's attention eagerly, delayed to the
                # collective's trigger so it covers the DMA burst.
                g = 0 if tb == 4 else 1
                emit_a2a(g)
                drive(emit_attention(tb, ps, pa, qt_tiles,
                                     delay_on=cc_last_write[g]), 10 ** 9)
                prev_gen, prev_steps = None, 0
            elif tb < NTB - 1:
                prev_gen = emit_attention(tb, ps, pa, qt_tiles)
                prev_steps = HPC * ((qb + 1) * 4 + 1)
            else:
                tail_qt = qt_tiles

        # ---- free projection-only pools; prefetch phase-3 data ------
        pa_cm.__exit__(None, None, None)
        ps_cm.__exit__(None, None, None)
        pp_cm.__exit__(None, None, None)
        for cm in reversed(trans):
            cm.__exit__(None, None, None)
        persistA_cm.__exit__(None, None, None)

        ap3_cm = tc.tile_pool(name="attn_sb", bufs=1)
        ap3 = ap3_cm.__enter__()
        wop_cm = tc.tile_pool(name="wop", bufs=8)
        wop = wop_cm.__enter__()
        ysb_cm = tc.tile_pool(name="ysb", bufs=2)
        ysbp = ysb_cm.__enter__()

        # wo(ob=0) + attn_sb loads fire as soon as the freed SBUF's WAR
        # deps clear (end of tb=7 projections) and run under the tail.
        wo_g0 = []
        for g in range(8):
            wt = wop.tile([128, 4, TB], F32R, name="wo_t", tag="wo")
            nc.sync.dma_start(
                wt[:], wo_d.rearrange("(hc p) n -> p hc n", p=128)
                [:, g * 4:(g + 1) * 4, 0:TB])
            wo_g0.append(wt)
        attn_sb = []
        for tc4 in range(3):
            t = ap3.tile([128, KC, 128], F32R, name=f"attn_sb{tc4}")
            if tc4 < 2:
                src = attn_g[0].ap().rearrange("(hc p) q -> p hc q", p=128)[
                    :, :, tc4 * 128:(tc4 + 1) * 128]
            else:
                src = attn_g[1].ap().rearrange("(hc p) q -> p hc q", p=128)
            nc.sync.dma_start(t[:], src)
            attn_sb.append(t)

        # ---- attention tail (tb=7) with generous psum buffering -----
        ps2_cm = tc.tile_pool(name="ps2", bufs=4, space="PSUM")
        ps2 = ps2_cm.__enter__()
        pa2_cm = tc.tile_pool(name="pa2", bufs=2, space="PSUM")
        pa2 = pa2_cm.__enter__()
        drive(emit_attention(NTB - 1, ps2, pa2, tail_qt), 10 ** 9)
        emit_a2a(2)
        t3 = ap3.tile([128, KC, 128], F32R, name="attn_sb3")
        nc.sync.dma_start(
            t3[:], attn_g[2].ap().rearrange("(hc p) q -> p hc q", p=128))
        attn_sb.append(t3)
        pa2_cm.__exit__(None, None, None)
        ps2_cm.__exit__(None, None, None)

        # ---- phase 3: y = attn_rows @ wo ----------------------------
        py_cm = tc.tile_pool(name="py", bufs=4, space="PSUM")
        pyp = py_cm.__enter__()
        for ob in range(8):
            if ob == 0:
                wo_g = wo_g0
            else:
                wo_g = []
                for g in range(8):
                    wt = wop.tile([128, 4, TB], F32R, name="wo_t", tag="wo")
                    nc.sync.dma_start(
                        wt[:], wo_d.rearrange("(hc p) n -> p hc n", p=128)
                        [:, g * 4:(g + 1) * 4, ob * TB:(ob + 1) * TB])
                    wo_g.append(wt)
            for tc4 in range(4):
                yps = pyp.tile([128, TB], F32, name="yps", tag="yps")
                for hc in range(KC):
                    nc.tensor.matmul(
                        yps[:], attn_sb[tc4][:, hc, :],
                        wo_g[hc // 4][:, hc % 4, :],
                        start=(hc == 0), stop=(hc == KC - 1),
                        skip_group_check=True)
                y_sb = ysbp.tile([128, TB], F32, name="y_sb", tag="y")
                nc.vector.tensor_copy(y_sb[:], yps[:])
                nc.sync.dma_start(
                    y_d[tc4 * 128:(tc4 + 1) * 128,
                        ob * TB:(ob + 1) * TB], y_sb[:])
        py_cm.__exit__(None, None, None)
        ysb_cm.__exit__(None, None, None)
        wop_cm.__exit__(None, None, None)
        ap3_cm.__exit__(None, None, None)
        for cm in reversed(tier2):
            cm.__exit__(None, None, None)
        persistB_cm.__exit__(None, None, None)

    nc.compile()
    return nc



_NC_CACHE = None


def _get_nc():
    global _NC_CACHE
    if _NC_CACHE is None:
        _NC_CACHE = build_attn_nc()
    return _NC_CACHE


def _host_reference(x, wq, wk, wv, wo, sincos, start_pos, causal_mask):
    """Numpy fallback (only used if the mask is not causal-tril)."""
    xq = (x @ wq).reshape(B, S, H, HD)
    xk = (x @ wk).reshape(B, S, KH, HD)
    xv = (x @ wv).reshape(B, S, KH, HD)
    sp = min(max(int(start_pos), 0), MS - S)
    sc = sincos[sp:sp + S]
    sin, cos = sc[:, :HD], sc[:, HD:]
    sin = sin[None, :, None, :]
    cos = cos[None, :, None, :]

    def rot(u):
        return np.concatenate([-u[..., HD // 2:], u[..., :HD // 2]], axis=-1)

    xq = xq * cos + rot(xq) * sin
    xk = xk * cos + rot(xk) * sin
    mask = np.broadcast_to(causal_mask[:, sp:sp + S, :MS], (B, S, MS))
    out = np.zeros((B, S, H, HD), dtype=np.float32)
    nrep = H // KH
    for b in range(B):
        for h in range(H):
            q = xq[b, :, h]
            k = xk[b, :, h // nrep]
            v = xv[b, :, h // nrep]
            s = (q @ k.T) * SCALE
            s = np.where(mask[b], s, -np.inf)
            s = s - s.max(axis=-1, keepdims=True)
            p = np.exp(s)
            p /= p.sum(axis=-1, keepdims=True)
            out[b, :, h] = p @ v
    return out.reshape(B, S, H * HD) @ wo


def kernel(x, wq, wk, wv, wo, cache_k, cache_v, sincos, causal_mask,
           start_pos):
    x = np.asarray(x, dtype=np.float32)
    wq = np.asarray(wq, dtype=np.float32)
    wk = np.asarray(wk, dtype=np.float32)
    wv = np.asarray(wv, dtype=np.float32)
    wo = np.asarray(wo, dtype=np.float32)
    sincos = np.asarray(sincos, dtype=np.float32)
    cm = np.asarray(causal_mask)
    sp = min(max(int(start_pos), 0), MS - S)

    tril = np.tril(np.ones((S, MS), dtype=bool))
    if not np.array_equal(cm[0, sp:sp + S, :], tril[:, :MS]):
        return _host_reference(x, wq, wk, wv, wo, sincos, start_pos,
                               cm).astype(np.float32)

    # host prep
    sc = sincos[sp:sp + S]
    sinT = np.ascontiguousarray(sc[:, :HD].T)       # [HD, S]
    cosT = np.ascontiguousarray(sc[:, HD:].T)       # [HD, S]
    xt = round_fp32r(np.ascontiguousarray(x.reshape(BS, D).T))
    wqs = wq * np.float32(SCALE)
    wo_r = round_fp32r(wo)

    maskd = np.zeros((128, 4, TB), dtype=np.float32)
    j = np.arange(128)[:, None, None]
    r = np.arange(4)[None, :, None]
    q = np.arange(TB)[None, None, :]
    maskd[(r * 128 + j) <= q] = 1.0

    rotm = np.zeros((HD, HD), dtype=np.float32)
    hh = HD // 2
    rotm[np.arange(hh) + hh, np.arange(hh)] = -1.0
    rotm[np.arange(hh), np.arange(hh) + hh] = 1.0

    ident = np.eye(128, dtype=np.float32)
    ones128 = np.ones((128, 128), dtype=np.float32)

    in_maps = []
    for c in range(N_CORES):
        in_maps.append({
            "xt": xt,
            "wq": round_fp32r(wqs[:, c * QF:(c + 1) * QF]),
            "wk": round_fp32r(wk[:, c * HD:(c + 1) * HD]),
            "wv": round_fp32r(wv[:, c * HD:(c + 1) * HD]),
            "wo": wo_r,
            "cosT": cosT, "sinT": sinT,
            "maskd": maskd, "rotm": rotm, "ident": ident,
            "ones128": ones128,
        })

    global _LAST_IN_MAPS
    _LAST_IN_MAPS = in_maps
    nc = _get_nc()
    res = run_bass_kernel_spmd(nc, in_maps, list(range(N_CORES)))
    # per-core y rows: [0:256] = b0 tokens c*256..; [256:384] = b1 tokens
    # c*128..; [384:512] = b1 tokens 1024+c*128..
    y = np.empty((BS, D), dtype=np.float32)
    for c in range(N_CORES):
        yc = res.results[c]["y"]
        y[c * 256:(c + 1) * 256] = yc[:256]
        y[S + c * 128:S + (c + 1) * 128] = yc[256:384]
        y[S + 1024 + c * 128:S + 1024 + (c + 1) * 128] = yc[384:]
    return y.reshape(B, S, D)



# revision 3
# speedup vs baseline: 1.2520x; 1.2520x over previous
"""Trainium2 Bass kernel for nn_Attention_13864154431876.

Dense transformer attention block: QKV projection + RoPE + causal GQA
attention (32 q heads, 8 kv heads, head_dim 128) + output projection.
B=2, S=2048, D=4096, start_pos=0 (cache fully overwritten).

Sharding (8 NeuronCores, tensor parallel by attention heads):
  - each core owns 4 q-heads and 1 kv-head (wq/wk/wv output-dim shards)
  - x is replicated (shipped pre-transposed as x^T so the contraction dim
    lands on partitions)
  - after attention, an on-chip AllToAll redistributes attn^T from
    head-sharded to token-sharded; each core then multiplies its 512-token
    slab against the full wo and the host concatenates the 8 slabs.

All on-chip data is bf16 (PSUM accumulation stays fp32); the 2e-2
relative-error budget has ample headroom for it, it halves all HBM/DMA
traffic and doubles DVE elementwise throughput.

Attention emission is software-pipelined: the AV matmul for chunk jc
is emitted one step after its score matmul so the exp (scalar engine)
latency never stalls the in-order tensor engine. The causal mask is
applied as a bias-accumulate matmul on the tensor engine (a triangular
-30000 tile added to the diagonal 128x128 sub-block of the scores) and
diagonal score/AV matmuls are shrunk to the live query subrange.
"""
import sys

sys.path.insert(0, "/root/.axon_site/_ro/trn_rl_repo")

import numpy as np
import ml_dtypes

import concourse.bass as bass
import concourse.mybir as mybir
import concourse.tile as tile
from concourse import bacc
from concourse.bass_utils import run_bass_kernel_spmd

F32 = mybir.dt.float32
BF16 = mybir.dt.bfloat16
AF = mybir.ActivationFunctionType
ALU = mybir.AluOpType

N_CORES = 8
B, S, D = 2, 2048, 4096
H, KH, HD = 32, 8, 128
MS = 2048                     # max_seq_len (cache length)
BS = B * S                    # flattened tokens, b-major
HPC = H // N_CORES            # q-heads per core = 4
QF = HPC * HD                 # per-core q-feature width = 512
TB = 512                      # token block
NTB = BS // TB                # 8 token blocks
QBPB = S // TB                # 4 q-blocks per batch element
KC = D // 128                 # 32 contraction chunks
JCB = S // 128                # 16 j-chunks per batch element
SCALE = 1.0 / np.sqrt(HD)
TOKS_PER_CORE = BS // N_CORES  # 512

BF16NP = ml_dtypes.bfloat16


def build_attn_nc(mock_collectives=False):
    nc = bacc.Bacc("TRN2", target_bir_lowering=False, debug=False,
                   num_devices=N_CORES)

    # ---- DRAM I/O ----------------------------------------------------
    xt_d = nc.dram_tensor("xt", [D, BS], BF16, kind="ExternalInput").ap()
    wq_d = nc.dram_tensor("wq", [D, QF], BF16, kind="ExternalInput").ap()
    wk_d = nc.dram_tensor("wk", [D, HD], BF16, kind="ExternalInput").ap()
    wv_d = nc.dram_tensor("wv", [D, HD], BF16, kind="ExternalInput").ap()
    wo_d = nc.dram_tensor("wo", [D, D], BF16, kind="ExternalInput").ap()
    cos_d = nc.dram_tensor("cosT", [HD, S], BF16, kind="ExternalInput").ap()
    sin_d = nc.dram_tensor("sinT", [HD, S], BF16, kind="ExternalInput").ap()
    maskb_d = nc.dram_tensor("maskb", [128, 128], BF16, kind="ExternalInput").ap()
    rot_d = nc.dram_tensor("rotm", [HD, HD], BF16, kind="ExternalInput").ap()
    ident_d = nc.dram_tensor("ident", [128, 128], BF16, kind="ExternalInput").ap()
    ones_d = nc.dram_tensor("ones128", [128, 128], BF16, kind="ExternalInput").ap()
    y_d = nc.dram_tensor("y", [TOKS_PER_CORE, D], F32, kind="ExternalOutput").ap()

    # internal DRAM for the per-batch-element AllToAlls.
    # attn_locX rows are chunk-major: chunk j (512 rows) = my 512 head-feats
    # for token group j. After A2A, attn_gX rows are global head-feats for
    # MY token slab.
    HTB = TB // 2  # 256
    QTB = TB // 4  # 128
    attn_loc = [nc.dram_tensor("attn_loc0", [BS, HTB], BF16),
                nc.dram_tensor("attn_loc1a", [BS, QTB], BF16),
                nc.dram_tensor("attn_loc1b", [BS, QTB], BF16)]
    attn_g = [nc.dram_tensor("attn_g0", [D, HTB], BF16),
              nc.dram_tensor("attn_g1a", [D, QTB], BF16),
              nc.dram_tensor("attn_g1b", [D, QTB], BF16)]

    with tile.TileContext(nc) as tc:
        # ---- pool stack (bottom-up; closed LIFO) --------------------
        persist0_cm = tc.tile_pool(name="persist0", bufs=1)
        persist0 = persist0_cm.__enter__()
        maskb_sb = persist0.tile([128, 128], BF16, name="maskb_sb")
        rot_sb = persist0.tile([HD, HD], BF16, name="rot_sb")
        ident_sb = persist0.tile([128, 128], BF16, name="ident_sb")
        ones_sb = persist0.tile([128, 128], BF16, name="ones_sb")
        kt_sb = [persist0.tile([HD, S], BF16, name=f"kt{b}_sb") for b in range(B)]
        v_sb = [persist0.tile([128, JCB, HD], BF16, name=f"v{b}_sb")
                for b in range(B)]
        attn_sb = [persist0.tile([128, KC, 128], BF16, name=f"attn_sb{i}")
                   for i in range(4)]

        # wo ring: slots exist from the start; DMAs are emitted late so the
        # loads don't compete with the startup xt/weight stream.
        wop_cm = tc.tile_pool(name="wop", bufs=14)
        wop = wop_cm.__enter__()

        tier2 = []

        def pool_t2(*a, **kw):
            cm = tc.tile_pool(*a, **kw)
            p = cm.__enter__()
            tier2.append(cm)
            return p

        qtp = pool_t2(name="qtp", bufs=5)
        tmpp = pool_t2(name="tmpp", bufs=2)
        ptp = pool_t2(name="ptp", bufs=4)
        denp = pool_t2(name="denp", bufs=2)
        recbp = pool_t2(name="recbp", bufs=2)
        atp = pool_t2(name="atp", bufs=2)
        asbp = pool_t2(name="asbp", bufs=2)

        persistA_cm = tc.tile_pool(name="persistA", bufs=1)
        persistA = persistA_cm.__enter__()
        wq_sb = persistA.tile([128, KC, QF], BF16, name="wq_sb")
        wk_sb = persistA.tile([128, KC, HD], BF16, name="wk_sb")
        wv_sb = persistA.tile([128, KC, HD], BF16, name="wv_sb")
        cos_sb = persistA.tile([HD, S], BF16, name="cos_sb")
        sin_sb = persistA.tile([HD, S], BF16, name="sin_sb")
        # chunked weight loads: first chunk lands fast so the first
        # projection matmuls are not stuck behind the full weight stream.
        wq_r = wq_d.rearrange("(kc p) n -> p kc n", p=128)
        nc.sync.dma_start(wq_sb[:, 0:4, :], wq_r[:, 0:4, :])
        nc.sync.dma_start(wk_sb[:], wk_d.rearrange("(kc p) n -> p kc n", p=128))
        nc.sync.dma_start(wv_sb[:], wv_d.rearrange("(kc p) n -> p kc n", p=128))
        nc.sync.dma_start(maskb_sb[:], maskb_d[:])
        nc.sync.dma_start(rot_sb[:], rot_d[:])
        nc.sync.dma_start(ident_sb[:], ident_d[:])
        nc.sync.dma_start(ones_sb[:], ones_d[:])

        trans = []

        def pool_tr(*a, **kw):
            cm = tc.tile_pool(*a, **kw)
            p = cm.__enter__()
            trans.append(cm)
            return p

        xtp = pool_tr(name="xtp", bufs=4)
        qrawp = pool_tr(name="qrawp", bufs=2)
        vtrawp = pool_tr(name="vtrawp", bufs=2)
        pp_cm = tc.tile_pool(name="pp", bufs=6, space="PSUM")
        pp = pp_cm.__enter__()
        ps_cm = tc.tile_pool(name="ps", bufs=1, space="PSUM")
        ps = ps_cm.__enter__()
        pa_cm = tc.tile_pool(name="pa", bufs=1, space="PSUM")
        pa = pa_cm.__enter__()

        def emit_attention(tb, ps_pool, pa_pool, qt_tiles, lag=1):
            """Generator: pipelined attention for token block tb.

            Per step: score(jc) [+mask bias], exp(jc), den(jc), then the AV
            matmul for chunk jc-lag — so the tensor engine never waits on
            the scalar-engine exp chain."""
            b, qb = tb // QBPB, tb % QBPB
            njc = (qb + 1) * 4
            grp = 0 if tb < QBPB else (1 if tb < 6 else 2)
            for h in range(HPC):
                denacc = denp.tile([128, TB], BF16, name="denacc", tag="den")
                aps = pa_pool.tile([128, TB], F32, name="aps", tag="aps")
                pending = []

                def emit_av(jc, pt, lo):
                    nc.tensor.matmul(
                        aps[:, lo:], v_sb[b][:, jc, :], pt[:, lo:],
                        start=(jc == 0), stop=(jc == njc - 1),
                        skip_group_check=True)

                for jc in range(njc):
                    r = jc - qb * 4
                    lo = max(r, 0) * 128
                    sps = ps_pool.tile([128, TB], F32, name="sps", tag="sps")
                    nc.tensor.matmul(
                        sps[:, lo:], kt_sb[b][:, jc * 128:(jc + 1) * 128],
                        qt_tiles[h][:, lo:], start=True, stop=(r < 0),
                        skip_group_check=True)
                    if r >= 0:
                        # causal bias: triangular -30000 on the diagonal
                        # 128x128 sub-block, accumulated on the PE
                        nc.tensor.matmul(
                            sps[:, lo:lo + 128], ident_sb[:], maskb_sb[:],
                            start=False, stop=True, skip_group_check=True)
                    pt = ptp.tile([128, TB], BF16, name="pt", tag="pt")
                    nc.scalar.activation(pt[:, lo:], sps[:, lo:], AF.Exp)
                    if jc == 0:
                        nc.vector.tensor_copy(denacc[:], pt[:])
                    else:
                        nc.vector.tensor_tensor(denacc[:, lo:], denacc[:, lo:],
                                                pt[:, lo:], ALU.add)
                    pending.append((jc, pt, lo))
                    while len(pending) > lag:
                        emit_av(*pending.pop(0))
                    yield
                while pending:
                    emit_av(*pending.pop(0))
                    yield
                # epilogue: colsum+broadcast via ones-matmul, fast
                # reciprocal, normalize; aps drained by the scalar engine
                asb = asbp.tile([128, TB], BF16, name="asb", tag="asb")
                nc.scalar.copy(asb[:], aps[:])
                denb = ps_pool.tile([128, TB], F32, name="denb", tag="sps")
                nc.tensor.matmul(denb[:], ones_sb[:], denacc[:],
                                 start=True, stop=True, skip_group_check=True)
                recipb = recbp.tile([128, TB], F32, name="recipb", tag="recb")
                nc.vector.reciprocal_approx_fast(recipb[:], denb[:])
                yield
                attn_t = atp.tile([128, TB], BF16, name="attn_t", tag="attn_t")
                nc.vector.tensor_tensor(attn_t[:], asb[:], recipb[:], ALU.mult)
                if grp == 0:
                    for half in range(2):
                        nc.sync.dma_start(
                            attn_loc[0].ap()[
                                (2 * tb + half) * 512 + h * 128:
                                (2 * tb + half) * 512 + (h + 1) * 128, :],
                            attn_t[:, half * HTB:(half + 1) * HTB])
                else:
                    lb = (tb - 4) % 2
                    for qt4 in range(4):
                        nc.sync.dma_start(
                            attn_loc[grp].ap()[
                                (4 * lb + qt4) * 512 + h * 128:
                                (4 * lb + qt4) * 512 + (h + 1) * 128, :],
                            attn_t[:, qt4 * QTB:(qt4 + 1) * QTB])
                yield

        def drive(gen, n):
            if gen is None:
                return None
            for _ in range(n):
                try:
                    next(gen)
                except StopIteration:
                    return None
            return gen

        def emit_a2a(g):
            if mock_collectives:
                nc.sync.dma_start(attn_g[g].ap()[:], attn_loc[g].ap()[:])
            else:
                nc.gpsimd.collective_compute(
                    "AllToAll", ALU.bypass,
                    replica_groups=[list(range(N_CORES))],
                    ins=[attn_loc[g].ap().opt()],
                    outs=[attn_g[g].ap().opt()],
                )

        def load_attn_sb(i):
            if i < 2:
                src = attn_g[0].ap().rearrange("(hc p) q -> p hc q", p=128)[
                    :, :, i * 128:(i + 1) * 128]
            else:
                src = attn_g[i - 1].ap().rearrange("(hc p) q -> p hc q", p=128)
            nc.sync.dma_start(attn_sb[i][:], src)

        wo_r = wo_d.rearrange("(hc p) n -> p hc n", p=128)

        def load_wo(ob):
            tiles = []
            for g in range(8):
                wt = wop.tile([128, 4, TB], BF16, name="wo_t", tag="wo")
                nc.sync.dma_start(
                    wt[:], wo_r[:, g * 4:(g + 1) * 4, ob * TB:(ob + 1) * TB])
                tiles.append(wt)
            return tiles

        prev_gen = None
        prev_steps = 0
        for tb in range(NTB):
            b, qb = tb // QBPB, tb % QBPB
            s0 = qb * TB
            per_kc = max(1, -(-prev_steps // KC))  # ceil
            # ---- projections for tb, interleaved with attention(tb-1)
            qps = [pp.tile([128, TB], F32, name=f"qps{h}", tag="proj")
                   for h in range(HPC)]
            kps = pp.tile([128, TB], F32, name="kps", tag="proj")
            vtps = pp.tile([128, TB], F32, name="vtps", tag="proj")
            for kc in range(KC):
                xt_t = xtp.tile([128, TB], BF16, name="xt_t", tag="xt")
                nc.sync.dma_start(
                    xt_t[:], xt_d[kc * 128:(kc + 1) * 128,
                                  tb * TB:(tb + 1) * TB])
                for h in range(HPC):
                    nc.tensor.matmul(
                        qps[h][:], wq_sb[:, kc, h * 128:(h + 1) * 128],
                        xt_t[:], start=(kc == 0), stop=(kc == KC - 1),
                        skip_group_check=True)
                nc.tensor.matmul(kps[:], wk_sb[:, kc, :], xt_t[:],
                                 start=(kc == 0), stop=(kc == KC - 1),
                                 skip_group_check=True)
                nc.tensor.matmul(vtps[:], wv_sb[:, kc, :], xt_t[:],
                                 start=(kc == 0), stop=(kc == KC - 1),
                                 skip_group_check=True)
                prev_gen = drive(prev_gen, per_kc)
                if tb == 0:
                    # stagger the remaining bulk loads behind the first
                    # x chunks so the pipeline starts immediately
                    if kc == 2:
                        nc.sync.dma_start(wq_sb[:, 4:12, :], wq_r[:, 4:12, :])
                    elif kc == 6:
                        nc.sync.dma_start(wq_sb[:, 12:24, :], wq_r[:, 12:24, :])
                    elif kc == 12:
                        nc.sync.dma_start(wq_sb[:, 24:32, :], wq_r[:, 24:32, :])
                    elif kc == 16:
                        nc.sync.dma_start(cos_sb[:], cos_d[:])
                        nc.sync.dma_start(sin_sb[:], sin_d[:])

            # ---- drains + RoPE + V transpose ------------------------
            qt_tiles = []
            for h in range(HPC):
                qraw = qrawp.tile([128, TB], BF16, name="qraw", tag="qraw")
                nc.scalar.copy(qraw[:], qps[h][:])
                rotps = ps.tile([128, TB], F32, name="rotps", tag="sps")
                nc.tensor.matmul(rotps[:], rot_sb[:], qraw[:],
                                 start=True, stop=True, skip_group_check=True)
                tcos = tmpp.tile([128, TB], BF16, name="tcos", tag="tmp")
                nc.vector.tensor_tensor(tcos[:], qraw[:],
                                        cos_sb[:, s0:s0 + TB], ALU.mult)
                tsin = tmpp.tile([128, TB], BF16, name="tsin", tag="tmp")
                nc.vector.tensor_tensor(tsin[:], rotps[:],
                                        sin_sb[:, s0:s0 + TB], ALU.mult)
                qt = qtp.tile([128, TB], BF16, name="qt", tag="qt")
                nc.vector.tensor_tensor(qt[:], tcos[:], tsin[:], ALU.add)
                qt_tiles.append(qt)
                prev_gen = drive(prev_gen, 1)
            # K
            kraw = qrawp.tile([128, TB], BF16, name="kraw", tag="qraw")
            nc.scalar.copy(kraw[:], kps[:])
            rotps = ps.tile([128, TB], F32, name="rotpsk", tag="sps")
            nc.tensor.matmul(rotps[:], rot_sb[:], kraw[:],
                             start=True, stop=True, skip_group_check=True)
            tcos = tmpp.tile([128, TB], BF16, name="tcosk", tag="tmp")
            nc.vector.tensor_tensor(tcos[:], kraw[:],
                                    cos_sb[:, s0:s0 + TB], ALU.mult)
            tsin = tmpp.tile([128, TB], BF16, name="tsink", tag="tmp")
            nc.vector.tensor_tensor(tsin[:], rotps[:],
                                    sin_sb[:, s0:s0 + TB], ALU.mult)
            nc.vector.tensor_tensor(kt_sb[b][:, s0:s0 + TB], tcos[:],
                                    tsin[:], ALU.add)
            # V: drain V^T then transpose 4x [128,128]
            vtraw = vtrawp.tile([128, TB], BF16, name="vtraw", tag="vtraw")
            nc.scalar.copy(vtraw[:], vtps[:])
            vtr = pp.tile([128, TB], BF16, name="vtr", tag="proj")
            for t4 in range(4):
                nc.tensor.transpose(vtr[:, t4 * 128:(t4 + 1) * 128],
                                    vtraw[:, t4 * 128:(t4 + 1) * 128],
                                    ident_sb[:])
            nc.vector.tensor_copy(
                v_sb[b].rearrange("p jc d -> p (jc d)")[:, s0:s0 + TB],
                vtr[:])
            prev_gen = drive(prev_gen, 10 ** 9)  # flush any leftovers
            if tb == 4:
                emit_a2a(0)
                load_attn_sb(0)
                load_attn_sb(1)
            elif tb == 6:
                emit_a2a(1)
                load_attn_sb(2)
            if tb < NTB - 1:
                prev_gen = emit_attention(tb, ps, pa, qt_tiles)
                prev_steps = HPC * ((qb + 1) * 4 + 3)
            else:
                tail_qt = qt_tiles

        # ---- free projection-only pools; prefetch wo(ob 0,1) --------
        pa_cm.__exit__(None, None, None)
        ps_cm.__exit__(None, None, None)
        pp_cm.__exit__(None, None, None)
        for cm in reversed(trans):
            cm.__exit__(None, None, None)
        persistA_cm.__exit__(None, None, None)

        wo_next = load_wo(0)

        # ---- attention tail (tb=7) with generous psum buffering -----
        ps2_cm = tc.tile_pool(name="ps2", bufs=4, space="PSUM")
        ps2 = ps2_cm.__enter__()
        pa2_cm = tc.tile_pool(name="pa2", bufs=2, space="PSUM")
        pa2 = pa2_cm.__enter__()
        drive(emit_attention(NTB - 1, ps2, pa2, tail_qt, lag=2), 10 ** 9)
        emit_a2a(2)
        load_attn_sb(3)
        pa2_cm.__exit__(None, None, None)
        ps2_cm.__exit__(None, None, None)
        for cm in reversed(tier2):
            cm.__exit__(None, None, None)

        ysb_cm = tc.tile_pool(name="ysb", bufs=4)
        ysbp = ysb_cm.__enter__()

        # ---- phase 3: y = attn_rows @ wo ----------------------------
        py_cm = tc.tile_pool(name="py", bufs=6, space="PSUM")
        pyp = py_cm.__enter__()
        for ob in range(8):
            wo_g = wo_next
            if ob < 7:
                wo_next = load_wo(ob + 1)
            for tc4 in range(4):
                yps = pyp.tile([128, TB], F32, name="yps", tag="yps")
                for hc in range(KC):
                    nc.tensor.matmul(
                        yps[:], attn_sb[tc4][:, hc, :],
                        wo_g[hc // 4][:, hc % 4, :],
                        start=(hc == 0), stop=(hc == KC - 1),
                        skip_group_check=True)
                y_sb = ysbp.tile([128, TB], F32, name="y_sb", tag="y")
                nc.vector.tensor_copy(y_sb[:], yps[:])
                nc.sync.dma_start(
                    y_d[tc4 * 128:(tc4 + 1) * 128,
                        ob * TB:(ob + 1) * TB], y_sb[:])
        py_cm.__exit__(None, None, None)
        ysb_cm.__exit__(None, None, None)
        wop_cm.__exit__(None, None, None)
        persist0_cm.__exit__(None, None, None)

    nc.compile()
    return nc


_NC_CACHE = None


def _get_nc():
    global _NC_CACHE
    if _NC_CACHE is None:
        _NC_CACHE = build_attn_nc()
    return _NC_CACHE


def _host_reference(x, wq, wk, wv, wo, sincos, start_pos, causal_mask):
    """Numpy fallback (only used if the mask is not causal-tril)."""
    xq = (x @ wq).reshape(B, S, H, HD)
    xk = (x @ wk).reshape(B, S, KH, HD)
    xv = (x @ wv).reshape(B, S, KH, HD)
    sp = min(max(int(start_pos), 0), MS - S)
    sc = sincos[sp:sp + S]
    sin, cos = sc[:, :HD], sc[:, HD:]
    sin = sin[None, :, None, :]
    cos = cos[None, :, None, :]

    def rot(u):
        return np.concatenate([-u[..., HD // 2:], u[..., :HD // 2]], axis=-1)

    xq = xq * cos + rot(xq) * sin
    xk = xk * cos + rot(xk) * sin
    mask = np.broadcast_to(causal_mask[:, sp:sp + S, :MS], (B, S, MS))
    out = np.zeros((B, S, H, HD), dtype=np.float32)
    nrep = H // KH
    for b in range(B):
        for h in range(H):
            q = xq[b, :, h]
            k = xk[b, :, h // nrep]
            v = xv[b, :, h // nrep]
            s = (q @ k.T) * SCALE
            s = np.where(mask[b], s, -np.inf)
            s = s - s.max(axis=-1, keepdims=True)
            p = np.exp(s)
            p /= p.sum(axis=-1, keepdims=True)
            out[b, :, h] = p @ v
    return out.reshape(B, S, H * HD) @ wo


def kernel(x, wq, wk, wv, wo, cache_k, cache_v, sincos, causal_mask,
           start_pos):
    x = np.asarray(x, dtype=np.float32)
    wq = np.asarray(wq, dtype=np.float32)
    wk = np.asarray(wk, dtype=np.float32)
    wv = np.asarray(wv, dtype=np.float32)
    wo = np.asarray(wo, dtype=np.float32)
    sincos = np.asarray(sincos, dtype=np.float32)
    cm = np.asarray(causal_mask)
    sp = min(max(int(start_pos), 0), MS - S)

    tril = np.tril(np.ones((S, MS), dtype=bool))
    if not np.array_equal(cm[0, sp:sp + S, :], tril[:, :MS]):
        return _host_reference(x, wq, wk, wv, wo, sincos, start_pos,
                               cm).astype(np.float32)

    # host prep
    sc = sincos[sp:sp + S]
    sinT = np.ascontiguousarray(sc[:, :HD].T).astype(BF16NP)   # [HD, S]
    cosT = np.ascontiguousarray(sc[:, HD:].T).astype(BF16NP)   # [HD, S]
    xt = np.ascontiguousarray(x.reshape(BS, D).T).astype(BF16NP)
    wqs = (wq * np.float32(SCALE)).astype(BF16NP)
    wo_b = wo.astype(BF16NP)

    # triangular causal bias for the diagonal 128x128 sub-block
    jj = np.arange(128)[:, None]
    qq = np.arange(128)[None, :]
    maskb = np.where(jj > qq, np.float32(-30000.0),
                     np.float32(0.0)).astype(BF16NP)

    rotm = np.zeros((HD, HD), dtype=np.float32)
    hh = HD // 2
    rotm[np.arange(hh) + hh, np.arange(hh)] = -1.0
    rotm[np.arange(hh), np.arange(hh) + hh] = 1.0

    ident = np.eye(128, dtype=np.float32).astype(BF16NP)
    ones128 = np.ones((128, 128), dtype=np.float32).astype(BF16NP)

    in_maps = []
    for c in range(N_CORES):
        in_maps.append({
            "xt": xt,
            "wq": np.ascontiguousarray(wqs[:, c * QF:(c + 1) * QF]),
            "wk": wk[:, c * HD:(c + 1) * HD].astype(BF16NP),
            "wv": wv[:, c * HD:(c + 1) * HD].astype(BF16NP),
            "wo": wo_b,
            "cosT": cosT, "sinT": sinT,
            "maskb": maskb, "rotm": rotm.astype(BF16NP), "ident": ident,
            "ones128": ones128,
        })

    global _LAST_IN_MAPS
    _LAST_IN_MAPS = in_maps
    nc = _get_nc()
    res = run_bass_kernel_spmd(nc, in_maps, list(range(N_CORES)))
    # per-core y rows: [0:256] = b0 tokens c*256..; [256:384] = b1 tokens
    # c*128..; [384:512] = b1 tokens 1024+c*128..
    y = np.empty((BS, D), dtype=np.float32)
    for c in range(N_CORES):
        yc = res.results[c]["y"]
        y[c * 256:(c + 1) * 256] = yc[:256]
        y[S + c * 128:S + (c + 1) * 128] = yc[256:384]
        y[S + 1024 + c * 128:S + 1024 + (c + 1) * 128] = yc[384:]
    return y.reshape(B, S, D)
